# revision 2
# baseline (speedup 1.0000x reference)
"""Trainium2 Bass kernel for a CNF (FFJORD-style) dynamics step.

Computes, for each sample z_i of a batch B=131072 (D=8, H=128):
    x  = concat([z_i, t])
    h1 = tanh(x @ W1 + b1)
    h2 = tanh(h1 @ W2 + b2)
    dz_dt   = h2 @ W3 + b3
    dlogp   = -trace(d dz_dt / d z_i)

The Jacobian trace has the closed form (u = 1-h1^2, v = 1-h2^2):
    trace = v . (u @ C)   with C[j,k] = W2[j,k] * (W3 @ W1[:D])[k,j]
so a single extra HxH matmul per sample replaces the full Jacobian.

Sharding: pure data parallel over 8 NeuronCores (batch split).
Layout on device is feature-major ([feature, batch] in SBUF partitions);
the host transposes z per shard and transposes dz_dt back.
"""

import numpy as np
import ml_dtypes

import concourse.bass as bass
import concourse.tile as tile
from concourse import bacc, mybir
from concourse.bass_utils import run_bass_kernel_spmd

BF16 = ml_dtypes.bfloat16

B = 131072
D = 8
H = 128
NCORES = 8
BC = B // NCORES          # samples per core
FD = 512                  # tile free-dim (samples per tile)
NTILES = BC // FD         # 32
GROUP = 4                 # tiles per PSUM output bank group

# test.py can read profiling info from here after calling kernel()
LAST_RESULTS = None


def _build_bass():
    nc = bacc.Bacc("TRN2", target_bir_lowering=False, debug=False,
                   num_devices=NCORES)
    f32 = mybir.dt.float32
    f32r = mybir.dt.float32r
    bf16 = mybir.dt.bfloat16

    zt_d = nc.dram_tensor("zt", [D, BC], f32r, kind="ExternalInput").ap()
    w1z_d = nc.dram_tensor("w1z", [D, H], f32r, kind="ExternalInput").ap()
    b1p_d = nc.dram_tensor("b1p", [H, 1], f32, kind="ExternalInput").ap()
    b2_d = nc.dram_tensor("b2", [H, 1], f32, kind="ExternalInput").ap()
    b3p_d = nc.dram_tensor("b3p", [H, 1], f32, kind="ExternalInput").ap()
    w2_d = nc.dram_tensor("w2", [H, H], bf16, kind="ExternalInput").ap()
    c_d = nc.dram_tensor("cmat", [H, H], bf16, kind="ExternalInput").ap()
    w3_d = nc.dram_tensor("w3", [H, D], bf16, kind="ExternalInput").ap()
    ones_d = nc.dram_tensor("onesneg", [H, 1], bf16, kind="ExternalInput").ap()

    dzt_d = nc.dram_tensor("dzt", [D, BC], f32, kind="ExternalOutput").ap()
    dlp_d = nc.dram_tensor("dlp", [1, BC], f32, kind="ExternalOutput").ap()

    mult = mybir.AluOpType.mult
    add = mybir.AluOpType.add
    Tanh = mybir.ActivationFunctionType.Tanh

    with tile.TileContext(nc) as tc:
        with (
            tc.tile_pool(name="wts", bufs=1) as wp,
            tc.tile_pool(name="io", bufs=3) as iop,
            tc.tile_pool(name="act", bufs=3) as ap_,
            tc.tile_pool(name="pmm", bufs=4, space="PSUM") as pmm,
            tc.tile_pool(name="pdz", bufs=2, space="PSUM") as pdz,
            tc.tile_pool(name="plp", bufs=2, space="PSUM") as plp,
        ):
            # resident weights
            w1z = wp.tile([D, H], f32r)
            nc.sync.dma_start(w1z[:], w1z_d)
            b1p = wp.tile([H, 1], f32)
            nc.sync.dma_start(b1p[:], b1p_d)
            b2 = wp.tile([H, 1], f32)
            nc.sync.dma_start(b2[:], b2_d)
            b3p = wp.tile([H, 1], f32)
            nc.sync.dma_start(b3p[:], b3p_d)
            w2 = wp.tile([H, H], bf16)
            nc.sync.dma_start(w2[:], w2_d)
            cm = wp.tile([H, H], bf16)
            nc.sync.dma_start(cm[:], c_d)
            w3 = wp.tile([H, D], bf16)
            nc.sync.dma_start(w3[:], w3_d)
            ones = wp.tile([H, 1], bf16)
            nc.sync.dma_start(ones[:], ones_d)

            dz_bank = None
            lp_bank = None
            for ti in range(NTILES):
                j = ti % GROUP
                cols = bass.ts(ti, FD)

                zt = iop.tile([D, FD], f32r, tag="zt")
                nc.sync.dma_start(zt[:], zt_d[:, cols])

                a1 = pmm.tile([H, FD], f32, tag="mm")
                nc.tensor.matmul(a1[:], w1z[:], zt[:])

                h1 = ap_.tile([H, FD], bf16, tag="h1")
                nc.scalar.activation(h1[:], a1[:], Tanh, bias=b1p[:])

                h1sq = ap_.tile([H, FD], bf16, tag="h1sq")
                nc.vector.tensor_tensor(h1sq[:], h1[:], h1[:], mult)
                u = ap_.tile([H, FD], bf16, tag="u")
                nc.vector.tensor_scalar(u[:], h1sq[:], -1.0, 1.0, mult, add)

                a2 = pmm.tile([H, FD], f32, tag="mm")
                nc.tensor.matmul(a2[:], w2[:], h1[:])
                h2 = ap_.tile([H, FD], bf16, tag="h2")
                nc.scalar.activation(h2[:], a2[:], Tanh, bias=b2[:])

                s = pmm.tile([H, FD], f32, tag="mm")
                nc.tensor.matmul(s[:], cm[:], u[:])

                h2sq = ap_.tile([H, FD], bf16, tag="h2sq")
                nc.vector.tensor_tensor(h2sq[:], h2[:], h2[:], mult)
                v = ap_.tile([H, FD], bf16, tag="v")
                nc.vector.tensor_scalar(v[:], h2sq[:], -1.0, 1.0, mult, add)

                p = ap_.tile([H, FD], bf16, tag="p")
                nc.vector.tensor_tensor(p[:], v[:], s[:], mult)

                if j == 0:
                    dz_bank = pdz.tile([H, FD], f32, tag="dzb")
                    lp_bank = plp.tile([H, FD], f32, tag="lpb")

                nc.tensor.matmul(dz_bank[32 * j:32 * j + D, :], w3[:], h2[:],
                                 tile_position=(0, 32 * j))
                nc.tensor.matmul(lp_bank[32 * j:32 * j + 1, :], ones[:], p[:],
                                 tile_position=(0, 32 * j))

                if j == GROUP - 1:
                    dzsb = iop.tile([H, FD], f32, tag="dzsb")
                    nc.vector.tensor_scalar(dzsb[:], dz_bank[:], b3p[:], None,
                                            add)
                    lpsb = iop.tile([H, FD], f32, tag="lpsb")
                    nc.scalar.copy(lpsb[:], lp_bank[:])
                    for jj in range(GROUP):
                        tjj = ti - (GROUP - 1) + jj
                        ocols = bass.ts(tjj, FD)
                        nc.sync.dma_start(dzt_d[:, ocols],
                                          dzsb[32 * jj:32 * jj + D, :])
                        nc.sync.dma_start(dlp_d[:, ocols],
                                          lpsb[32 * jj:32 * jj + 1, :])
    nc.compile()
    return nc


_NC_CACHE = None


def kernel(z, logp_z, t, W1, b1, W2, b2, W3, b3):
    global LAST_RESULTS, _NC_CACHE

    z = np.ascontiguousarray(np.asarray(z, dtype=np.float32))
    t_s = float(np.asarray(t, dtype=np.float32).reshape(-1)[0])
    W1 = np.asarray(W1, dtype=np.float32)
    b1 = np.asarray(b1, dtype=np.float32)
    W2 = np.asarray(W2, dtype=np.float32)
    b2 = np.asarray(b2, dtype=np.float32)
    W3 = np.asarray(W3, dtype=np.float32)
    b3 = np.asarray(b3, dtype=np.float32)

    W1z = np.ascontiguousarray(W1[:D, :])              # [D, H]
    b1p = (b1 + t_s * W1[D, :]).reshape(H, 1).astype(np.float32)
    M = W3.astype(np.float64) @ W1z.astype(np.float64)  # [H, H]
    C = (W2.astype(np.float64) * M.T).astype(np.float32)
    w2_bf = W2.astype(BF16)
    c_bf = C.astype(BF16)
    w3_bf = np.ascontiguousarray(W3.astype(BF16))
    ones_neg = np.full((H, 1), -1.0, dtype=BF16)
    b2v = b2.reshape(H, 1).astype(np.float32)
    b3p = np.zeros((H, 1), dtype=np.float32)
    for jj in range(GROUP):
        b3p[32 * jj:32 * jj + D, 0] = b3

    if _NC_CACHE is None:
        _NC_CACHE = _build_bass()
    nc = _NC_CACHE

    in_maps = []
    for c in range(NCORES):
        zt = np.ascontiguousarray(z[c * BC:(c + 1) * BC, :].T)
        in_maps.append({
            "zt": zt,
            "w1z": W1z,
            "b1p": b1p,
            "b2": b2v,
            "b3p": b3p,
            "w2": w2_bf,
            "cmat": c_bf,
            "w3": w3_bf,
            "onesneg": ones_neg,
        })

    res = run_bass_kernel_spmd(nc, in_maps, core_ids=list(range(NCORES)))
    LAST_RESULTS = res

    dz = np.empty((B, D), dtype=np.float32)
    dlogp = np.empty((B, 1), dtype=np.float32)
    for c in range(NCORES):
        dz[c * BC:(c + 1) * BC, :] = res.results[c]["dzt"].T
        dlogp[c * BC:(c + 1) * BC, 0] = res.results[c]["dlp"].reshape(-1)
    return (dz, dlogp)


# revision 3
# speedup vs baseline: 1.2696x; 1.2696x over previous
"""Trainium2 Bass kernel for a CNF (FFJORD-style) dynamics step.

Computes, for each sample z_i of a batch B=131072 (D=8, H=128):
    x  = concat([z_i, t])
    h1 = tanh(x @ W1 + b1)
    h2 = tanh(h1 @ W2 + b2)
    dz_dt   = h2 @ W3 + b3
    dlogp   = -trace(d dz_dt / d z_i)

The Jacobian trace has the closed form (u = 1-h1^2, v = 1-h2^2):
    trace = v . (u @ C)   with C[j,k] = W2[j,k] * (W3 @ W1[:D])[k,j]
so a single extra HxH matmul per sample replaces the full Jacobian.
On device the "1 -" is folded into the matmul: s'' = (-C)^T h1sq and
p = (s'' + c0) * v with c0 = C^T 1 applied as a fused per-partition
scalar in one DVE scalar_tensor_tensor op.

Sharding: pure data parallel over 8 NeuronCores (batch split).
Layout on device is feature-major ([feature, batch] in SBUF partitions);
the host transposes z per shard and transposes dz_dt back.
"""

import numpy as np
import ml_dtypes

import concourse.bass as bass
import concourse.tile as tile
from concourse import bacc, mybir
from concourse.bass_utils import run_bass_kernel_spmd

BF16 = ml_dtypes.bfloat16

B = 131072
D = 8
H = 128
NCORES = 8
BC = B // NCORES          # samples per core
FD = 512                  # tile free-dim (samples per tile)
NTILES = BC // FD         # 32
GROUP = 2                 # tiles per PSUM output bank group

# test.py can read profiling info from here after calling kernel()
LAST_RESULTS = None


def _build_bass():
    nc = bacc.Bacc("TRN2", target_bir_lowering=False, debug=False,
                   num_devices=NCORES)
    f32 = mybir.dt.float32
    bf16 = mybir.dt.bfloat16

    zt_d = nc.dram_tensor("zt", [D, BC], bf16, kind="ExternalInput").ap()
    w1z_d = nc.dram_tensor("w1z", [D, H], bf16, kind="ExternalInput").ap()
    b1p_d = nc.dram_tensor("b1p", [H, 1], f32, kind="ExternalInput").ap()
    b2_d = nc.dram_tensor("b2", [H, 1], f32, kind="ExternalInput").ap()
    b3p_d = nc.dram_tensor("b3p", [H, 1], f32, kind="ExternalInput").ap()
    c0_d = nc.dram_tensor("c0", [H, 1], f32, kind="ExternalInput").ap()
    w2_d = nc.dram_tensor("w2", [H, H], bf16, kind="ExternalInput").ap()
    cn_d = nc.dram_tensor("cneg", [H, H], bf16, kind="ExternalInput").ap()
    w3_d = nc.dram_tensor("w3", [H, D], bf16, kind="ExternalInput").ap()
    ones_d = nc.dram_tensor("onesneg", [H, 1], bf16, kind="ExternalInput").ap()

    dzt_d = nc.dram_tensor("dzt", [D, BC], f32, kind="ExternalOutput").ap()
    dlp_d = nc.dram_tensor("dlp", [1, BC], f32, kind="ExternalOutput").ap()

    mult = mybir.AluOpType.mult
    add = mybir.AluOpType.add
    Tanh = mybir.ActivationFunctionType.Tanh

    with tile.TileContext(nc) as tc:
        with (
            tc.tile_pool(name="wts", bufs=1) as wp,
            tc.tile_pool(name="io", bufs=3) as iop,
            tc.tile_pool(name="act", bufs=3) as ap_,
            tc.tile_pool(name="pa1", bufs=2, space="PSUM") as pa1,
            tc.tile_pool(name="pa2", bufs=2, space="PSUM") as pa2,
            tc.tile_pool(name="psm", bufs=2, space="PSUM") as psm,
            tc.tile_pool(name="pout", bufs=2, space="PSUM") as pout,
        ):
            # resident weights
            w1z = wp.tile([D, H], bf16)
            nc.sync.dma_start(w1z[:], w1z_d)
            b1p = wp.tile([H, 1], f32)
            nc.sync.dma_start(b1p[:], b1p_d)
            b2 = wp.tile([H, 1], f32)
            nc.sync.dma_start(b2[:], b2_d)
            b3p = wp.tile([H, 1], f32)
            nc.sync.dma_start(b3p[:], b3p_d)
            c0 = wp.tile([H, 1], f32)
            nc.sync.dma_start(c0[:], c0_d)
            w2 = wp.tile([H, H], bf16)
            nc.sync.dma_start(w2[:], w2_d)
            cn = wp.tile([H, H], bf16)
            nc.sync.dma_start(cn[:], cn_d)
            w3 = wp.tile([H, D], bf16)
            nc.sync.dma_start(w3[:], w3_d)
            ones = wp.tile([H, 1], bf16)
            nc.sync.dma_start(ones[:], ones_d)

            out_bank = None
            for ti in range(NTILES):
                j = ti % GROUP
                cols = bass.ts(ti, FD)

                zt = iop.tile([D, FD], bf16, tag="zt")
                nc.sync.dma_start(zt[:], zt_d[:, cols])

                a1 = pa1.tile([H, FD], f32, tag="a1")
                nc.tensor.matmul(a1[:], w1z[:], zt[:])

                h1 = ap_.tile([H, FD], bf16, tag="h1")
                nc.scalar.activation(h1[:], a1[:], Tanh, bias=b1p[:])

                h1sq = ap_.tile([H, FD], bf16, tag="h1sq")
                nc.vector.tensor_tensor(h1sq[:], h1[:], h1[:], mult)

                a2 = pa2.tile([H, FD], f32, tag="a2")
                nc.tensor.matmul(a2[:], w2[:], h1[:])
                h2 = ap_.tile([H, FD], bf16, tag="h2")
                nc.scalar.activation(h2[:], a2[:], Tanh, bias=b2[:])

                s = psm.tile([H, FD], f32, tag="s")
                nc.tensor.matmul(s[:], cn[:], h1sq[:])

                h2sq = ap_.tile([H, FD], bf16, tag="h2sq")
                nc.vector.tensor_tensor(h2sq[:], h2[:], h2[:], mult)
                v = ap_.tile([H, FD], bf16, tag="v")
                nc.vector.tensor_scalar(v[:], h2sq[:], -1.0, 1.0, mult, add)

                # p = (s + c0) * v  in one fused DVE op
                p = ap_.tile([H, FD], bf16, tag="p")
                nc.vector.scalar_tensor_tensor(p[:], s[:], c0[:], v[:],
                                               add, mult)

                if j == 0:
                    out_bank = pout.tile([H, FD], f32, tag="ob")

                nc.tensor.matmul(out_bank[32 * j:32 * j + D, :], w3[:], h2[:],
                                 tile_position=(0, 32 * j))
                nc.tensor.matmul(out_bank[64 + 32 * j:64 + 32 * j + 1, :],
                                 ones[:], p[:],
                                 tile_position=(0, 64 + 32 * j))

                if j == GROUP - 1:
                    osb = iop.tile([H, FD], f32, tag="osb")
                    nc.vector.tensor_scalar(osb[:], out_bank[:], b3p[:], None,
                                            add)
                    for jj in range(GROUP):
                        tjj = ti - (GROUP - 1) + jj
                        ocols = bass.ts(tjj, FD)
                        nc.sync.dma_start(dzt_d[:, ocols],
                                          osb[32 * jj:32 * jj + D, :])
                        nc.sync.dma_start(dlp_d[:, ocols],
                                          osb[64 + 32 * jj:64 + 32 * jj + 1, :])
    nc.compile()
    return nc


_NC_CACHE = None


def kernel(z, logp_z, t, W1, b1, W2, b2, W3, b3):
    global LAST_RESULTS, _NC_CACHE

    z = np.ascontiguousarray(np.asarray(z, dtype=np.float32))
    t_s = float(np.asarray(t, dtype=np.float32).reshape(-1)[0])
    W1 = np.asarray(W1, dtype=np.float32)
    b1 = np.asarray(b1, dtype=np.float32)
    W2 = np.asarray(W2, dtype=np.float32)
    b2 = np.asarray(b2, dtype=np.float32)
    W3 = np.asarray(W3, dtype=np.float32)
    b3 = np.asarray(b3, dtype=np.float32)

    W1z = np.ascontiguousarray(W1[:D, :])              # [D, H]
    b1p = (b1 + t_s * W1[D, :]).reshape(H, 1).astype(np.float32)
    M = W3.astype(np.float64) @ W1z.astype(np.float64)  # [H, H]
    C = W2.astype(np.float64) * M.T                     # [H(j), H(k)]
    c0 = C.sum(axis=0).reshape(H, 1).astype(np.float32)  # C^T @ 1
    w1z_bf = W1z.astype(BF16)
    w2_bf = W2.astype(BF16)
    cn_bf = (-C).astype(np.float32).astype(BF16)
    w3_bf = np.ascontiguousarray(W3.astype(BF16))
    ones_neg = np.full((H, 1), -1.0, dtype=BF16)
    b2v = b2.reshape(H, 1).astype(np.float32)
    b3p = np.zeros((H, 1), dtype=np.float32)
    for jj in range(GROUP):
        b3p[32 * jj:32 * jj + D, 0] = b3

    if _NC_CACHE is None:
        _NC_CACHE = _build_bass()
    nc = _NC_CACHE

    in_maps = []
    for c in range(NCORES):
        zt = np.ascontiguousarray(z[c * BC:(c + 1) * BC, :].T.astype(BF16))
        in_maps.append({
            "zt": zt,
            "w1z": w1z_bf,
            "b1p": b1p,
            "b2": b2v,
            "b3p": b3p,
            "c0": c0,
            "w2": w2_bf,
            "cneg": cn_bf,
            "w3": w3_bf,
            "onesneg": ones_neg,
        })

    res = run_bass_kernel_spmd(nc, in_maps, core_ids=list(range(NCORES)))
    LAST_RESULTS = res

    dz = np.empty((B, D), dtype=np.float32)
    dlogp = np.empty((B, 1), dtype=np.float32)
    for c in range(NCORES):
        dz[c * BC:(c + 1) * BC, :] = res.results[c]["dzt"].T
        dlogp[c * BC:(c + 1) * BC, 0] = res.results[c]["dlp"].reshape(-1)
    return (dz, dlogp)


# revision 4
# speedup vs baseline: 1.2980x; 1.0223x over previous
"""Trainium2 Bass kernel for a CNF (FFJORD-style) dynamics step.

Computes, for each sample z_i of a batch B=131072 (D=8, H=128):
    x  = concat([z_i, t])
    h1 = tanh(x @ W1 + b1)
    h2 = tanh(h1 @ W2 + b2)
    dz_dt   = h2 @ W3 + b3
    dlogp   = -trace(d dz_dt / d z_i)

The Jacobian trace has the closed form (u = 1-h1^2, v = 1-h2^2):
    trace = v . (u @ C)   with C[j,k] = W2[j,k] * (W3 @ W1[:D])[k,j]
so a single extra HxH matmul per sample replaces the full Jacobian.
On device the "1 -" is folded into the matmul: s'' = (-C)^T h1sq and
p = (s'' + c0) * v with c0 = C^T 1 applied as a fused per-partition
scalar in one DVE scalar_tensor_tensor op.

Sharding: pure data parallel over 8 NeuronCores (batch split).
Layout on device is feature-major ([feature, batch] in SBUF partitions);
the host transposes z per shard and transposes dz_dt back.

PE-array packing: the K=8 layer-1 matmuls of two consecutive tiles run
concurrently in different 32-row groups; the M=8 dz matmul and the M=1
trace matmul run concurrently in different 32-col groups of one shared
PSUM output bank (dz rows 0-7/32-39, dlogp rows 64/96).
"""

import numpy as np
import ml_dtypes

import concourse.bass as bass
import concourse.tile as tile
from concourse import bacc, mybir
from concourse.bass_utils import run_bass_kernel_spmd

BF16 = ml_dtypes.bfloat16

B = 131072
D = 8
H = 128
NCORES = 8
BC = B // NCORES          # samples per core
FD = 512                  # tile free-dim (samples per tile)
NTILES = BC // FD         # 32
GROUP = 2                 # tiles per PSUM output bank group

# test.py can read profiling info from here after calling kernel()
LAST_RESULTS = None


def _build_bass(with_b3):
    nc = bacc.Bacc("TRN2", target_bir_lowering=False, debug=False,
                   num_devices=NCORES)
    f32 = mybir.dt.float32
    bf16 = mybir.dt.bfloat16

    zt_d = nc.dram_tensor("zt", [D, BC], bf16, kind="ExternalInput").ap()
    w1r_d = nc.dram_tensor("w1r", [32 + D, H], bf16, kind="ExternalInput").ap()
    b1p_d = nc.dram_tensor("b1p", [H, 1], f32, kind="ExternalInput").ap()
    b2_d = nc.dram_tensor("b2", [H, 1], f32, kind="ExternalInput").ap()
    b3p_d = nc.dram_tensor("b3p", [H, 1], f32, kind="ExternalInput").ap()
    c0_d = nc.dram_tensor("c0", [H, 1], f32, kind="ExternalInput").ap()
    w2_d = nc.dram_tensor("w2", [H, H], bf16, kind="ExternalInput").ap()
    cn_d = nc.dram_tensor("cneg", [H, H], bf16, kind="ExternalInput").ap()
    w3_d = nc.dram_tensor("w3", [H, D], bf16, kind="ExternalInput").ap()
    ones_d = nc.dram_tensor("onesneg", [H, 1], bf16, kind="ExternalInput").ap()

    dzt_d = nc.dram_tensor("dzt", [D, BC], f32, kind="ExternalOutput").ap()
    dlp_d = nc.dram_tensor("dlp", [1, BC], f32, kind="ExternalOutput").ap()

    mult = mybir.AluOpType.mult
    add = mybir.AluOpType.add
    Tanh = mybir.ActivationFunctionType.Tanh
    Square = mybir.ActivationFunctionType.Square

    with tile.TileContext(nc) as tc:
        with (
            tc.tile_pool(name="wts", bufs=1) as wp,
            tc.tile_pool(name="io", bufs=4) as iop,
            tc.tile_pool(name="act", bufs=4) as ap_,
            tc.tile_pool(name="pa1", bufs=2, space="PSUM") as pa1,
            tc.tile_pool(name="pa2", bufs=2, space="PSUM") as pa2,
            tc.tile_pool(name="psm", bufs=2, space="PSUM") as psm,
            tc.tile_pool(name="pout", bufs=2, space="PSUM") as pout,
        ):
            # resident weights
            w1r = wp.tile([32 + D, H], bf16)
            nc.sync.dma_start(w1r[:], w1r_d)
            b1p = wp.tile([H, 1], f32)
            nc.sync.dma_start(b1p[:], b1p_d)
            b2 = wp.tile([H, 1], f32)
            nc.sync.dma_start(b2[:], b2_d)
            b3p = wp.tile([H, 1], f32)
            nc.sync.dma_start(b3p[:], b3p_d)
            c0 = wp.tile([H, 1], f32)
            nc.sync.dma_start(c0[:], c0_d)
            w2 = wp.tile([H, H], bf16)
            nc.sync.dma_start(w2[:], w2_d)
            cn = wp.tile([H, H], bf16)
            nc.sync.dma_start(cn[:], cn_d)
            w3 = wp.tile([H, D], bf16)
            nc.sync.dma_start(w3[:], w3_d)
            ones = wp.tile([H, 1], bf16)
            nc.sync.dma_start(ones[:], ones_d)

            for tg in range(NTILES // GROUP):
                t0 = tg * GROUP
                colset = [bass.ts(t0 + j, FD) for j in range(GROUP)]

                # paired z load: tile t0 at partitions 0-7, t0+1 at 32-39
                ztp = iop.tile([32 + D, FD], bf16, tag="zt")
                for j in range(GROUP):
                    nc.sync.dma_start(ztp[32 * j:32 * j + D, :],
                                      zt_d[:, colset[j]])

                # layer-1 matmuls of both tiles, packed in row groups 0/1
                a1s = []
                for j in range(GROUP):
                    a1 = pa1.tile([H, FD], f32, tag="a1", name=f"a1_{tg}_{j}")
                    nc.tensor.matmul(a1[:], w1r[32 * j:32 * j + D, :],
                                     ztp[32 * j:32 * j + D, :],
                                     tile_position=(32 * j, 0))
                    a1s.append(a1)

                h1s, h1sqs = [], []
                for j in range(GROUP):
                    h1 = ap_.tile([H, FD], bf16, tag="h1", name=f"h1_{tg}_{j}")
                    nc.scalar.activation(h1[:], a1s[j][:], Tanh, bias=b1p[:])
                    h1s.append(h1)
                for j in range(GROUP):
                    h1sq = ap_.tile([H, FD], bf16, tag="h1sq",
                                    name=f"h1sq_{tg}_{j}")
                    nc.vector.tensor_tensor(h1sq[:], h1s[j][:], h1s[j][:],
                                            mult)
                    h1sqs.append(h1sq)

                a2s, ss = [], []
                for j in range(GROUP):
                    a2 = pa2.tile([H, FD], f32, tag="a2", name=f"a2_{tg}_{j}")
                    nc.tensor.matmul(a2[:], w2[:], h1s[j][:])
                    a2s.append(a2)
                    s = psm.tile([H, FD], f32, tag="s", name=f"s_{tg}_{j}")
                    nc.tensor.matmul(s[:], cn[:], h1sqs[j][:])
                    ss.append(s)

                h2s = []
                for j in range(GROUP):
                    h2 = ap_.tile([H, FD], bf16, tag="h2", name=f"h2_{tg}_{j}")
                    nc.scalar.activation(h2[:], a2s[j][:], Tanh, bias=b2[:])
                    h2s.append(h2)

                ps = []
                for j in range(GROUP):
                    h2sq = ap_.tile([H, FD], bf16, tag="h2sq",
                                    name=f"h2sq_{tg}_{j}")
                    nc.scalar.activation(h2sq[:], h2s[j][:], Square)
                    v = ap_.tile([H, FD], bf16, tag="v", name=f"v_{tg}_{j}")
                    nc.vector.tensor_scalar(v[:], h2sq[:], -1.0, 1.0, mult,
                                            add)
                    p = ap_.tile([H, FD], bf16, tag="p", name=f"p_{tg}_{j}")
                    nc.vector.scalar_tensor_tensor(p[:], ss[j][:], c0[:],
                                                   v[:], add, mult)
                    ps.append(p)

                out_bank = pout.tile([H, FD], f32, tag="ob")
                for j in range(GROUP):
                    nc.tensor.matmul(out_bank[32 * j:32 * j + D, :], w3[:],
                                     h2s[j][:], tile_position=(0, 32 * j))
                    nc.tensor.matmul(out_bank[64 + 32 * j:64 + 32 * j + 1, :],
                                     ones[:], ps[j][:],
                                     tile_position=(0, 64 + 32 * j))

                osb = iop.tile([H, FD], f32, tag="osb")
                if with_b3:
                    nc.vector.tensor_scalar(osb[:], out_bank[:], b3p[:], None,
                                            add)
                else:
                    nc.vector.tensor_copy(osb[:], out_bank[:])
                for j in range(GROUP):
                    nc.sync.dma_start(dzt_d[:, colset[j]],
                                      osb[32 * j:32 * j + D, :])
                    nc.sync.dma_start(dlp_d[:, colset[j]],
                                      osb[64 + 32 * j:64 + 32 * j + 1, :])
    nc.compile()
    return nc


_NC_CACHE = {}


def kernel(z, logp_z, t, W1, b1, W2, b2, W3, b3):
    global LAST_RESULTS, _NC_CACHE

    z = np.ascontiguousarray(np.asarray(z, dtype=np.float32))
    t_s = float(np.asarray(t, dtype=np.float32).reshape(-1)[0])
    W1 = np.asarray(W1, dtype=np.float32)
    b1 = np.asarray(b1, dtype=np.float32)
    W2 = np.asarray(W2, dtype=np.float32)
    b2 = np.asarray(b2, dtype=np.float32)
    W3 = np.asarray(W3, dtype=np.float32)
    b3 = np.asarray(b3, dtype=np.float32)

    W1z = np.ascontiguousarray(W1[:D, :])              # [D, H]
    b1p = (b1 + t_s * W1[D, :]).reshape(H, 1).astype(np.float32)
    M = W3.astype(np.float64) @ W1z.astype(np.float64)  # [H, H]
    C = W2.astype(np.float64) * M.T                     # [H(j), H(k)]
    c0 = C.sum(axis=0).reshape(H, 1).astype(np.float32)  # C^T @ 1
    w1r = np.zeros((32 + D, H), dtype=BF16)
    for j in range(GROUP):
        w1r[32 * j:32 * j + D, :] = W1z.astype(BF16)
    w2_bf = W2.astype(BF16)
    cn_bf = (-C).astype(np.float32).astype(BF16)
    w3_bf = np.ascontiguousarray(W3.astype(BF16))
    ones_neg = np.full((H, 1), -1.0, dtype=BF16)
    b2v = b2.reshape(H, 1).astype(np.float32)
    b3p = np.zeros((H, 1), dtype=np.float32)
    for jj in range(GROUP):
        b3p[32 * jj:32 * jj + D, 0] = b3
    with_b3 = bool(np.any(b3 != 0))

    if with_b3 not in _NC_CACHE:
        _NC_CACHE[with_b3] = _build_bass(with_b3)
    nc = _NC_CACHE[with_b3]

    in_maps = []
    for c in range(NCORES):
        zt = np.ascontiguousarray(z[c * BC:(c + 1) * BC, :].T.astype(BF16))
        in_maps.append({
            "zt": zt,
            "w1r": w1r,
            "b1p": b1p,
            "b2": b2v,
            "b3p": b3p,
            "c0": c0,
            "w2": w2_bf,
            "cneg": cn_bf,
            "w3": w3_bf,
            "onesneg": ones_neg,
        })

    res = run_bass_kernel_spmd(nc, in_maps, core_ids=list(range(NCORES)))
    LAST_RESULTS = res

    dz = np.empty((B, D), dtype=np.float32)
    dlogp = np.empty((B, 1), dtype=np.float32)
    for c in range(NCORES):
        dz[c * BC:(c + 1) * BC, :] = res.results[c]["dzt"].T
        dlogp[c * BC:(c + 1) * BC, 0] = res.results[c]["dlp"].reshape(-1)
    return (dz, dlogp)


# revision 5
# speedup vs baseline: 1.4944x; 1.1513x over previous
"""Trainium2 Bass kernel for a CNF (FFJORD-style) dynamics step.

Computes, for each sample z_i of a batch B=131072 (D=8, H=128):
    x  = concat([z_i, t])
    h1 = tanh(x @ W1 + b1)
    h2 = tanh(h1 @ W2 + b2)
    dz_dt   = h2 @ W3 + b3
    dlogp   = -trace(d dz_dt / d z_i)

The Jacobian trace has the closed form (u = 1-h1^2, v = 1-h2^2):
    trace = v . (u @ C)   with C[j,k] = W2[j,k] * (W3 @ W1[:D])[k,j]
so a single extra HxH matmul per sample replaces the full Jacobian.
On device the "1 -" is folded into the matmul: s'' = (-C)^T h1sq and
p = (s'' + c0) * v with c0 = C^T 1 applied as a fused per-partition
scalar in one DVE scalar_tensor_tensor op.

Sharding: pure data parallel over 8 NeuronCores (batch split).
Layout on device is feature-major ([feature, batch] in SBUF partitions);
the host transposes z per shard and transposes dz_dt back.

PE-array packing: the K=8 layer-1 matmuls of two consecutive tiles run
concurrently in different 32-row groups; the M=8 dz matmul and the M=1
trace matmul run concurrently in different 32-col groups of one shared
PSUM output bank (dz rows 0-7/32-39, dlogp rows 64/96).
"""

import numpy as np
import ml_dtypes

import concourse.bass as bass
import concourse.tile as tile
from concourse import bacc, mybir
from concourse.bass_utils import run_bass_kernel_spmd

BF16 = ml_dtypes.bfloat16

B = 131072
D = 8
H = 128
NCORES = 8
BC = B // NCORES          # samples per core
FD = 512                  # tile free-dim (samples per tile)
NTILES = BC // FD         # 32
GROUP = 2                 # tiles per PSUM output bank group

# test.py can read profiling info from here after calling kernel()
LAST_RESULTS = None


def _build_bass(with_b3):
    nc = bacc.Bacc("TRN2", target_bir_lowering=False, debug=False,
                   num_devices=NCORES)
    f32 = mybir.dt.float32
    bf16 = mybir.dt.bfloat16

    zt_d = nc.dram_tensor("zt", [D, BC], bf16, kind="ExternalInput").ap()
    w1r_d = nc.dram_tensor("w1r", [32 + D, H], bf16, kind="ExternalInput").ap()
    b1p_d = nc.dram_tensor("b1p", [H, 1], f32, kind="ExternalInput").ap()
    b2_d = nc.dram_tensor("b2", [H, 1], f32, kind="ExternalInput").ap()
    b3p_d = nc.dram_tensor("b3p", [H, 1], f32, kind="ExternalInput").ap()
    c0_d = nc.dram_tensor("c0", [H, 1], f32, kind="ExternalInput").ap()
    w2_d = nc.dram_tensor("w2", [H, H], bf16, kind="ExternalInput").ap()
    cn_d = nc.dram_tensor("cneg", [H, H], bf16, kind="ExternalInput").ap()
    w3_d = nc.dram_tensor("w3", [H, D], bf16, kind="ExternalInput").ap()
    ones_d = nc.dram_tensor("onesneg", [H, 1], bf16, kind="ExternalInput").ap()

    dzt_d = nc.dram_tensor("dzt", [D, BC], f32, kind="ExternalOutput").ap()
    dlp_d = nc.dram_tensor("dlp", [1, BC], f32, kind="ExternalOutput").ap()

    mult = mybir.AluOpType.mult
    add = mybir.AluOpType.add
    Tanh = mybir.ActivationFunctionType.Tanh
    Square = mybir.ActivationFunctionType.Square

    with tile.TileContext(nc) as tc:
        with (
            tc.tile_pool(name="wts", bufs=1) as wp,
            tc.tile_pool(name="io", bufs=6) as iop,
            tc.tile_pool(name="act", bufs=6) as ap_,
            tc.tile_pool(name="pmm", bufs=6, space="PSUM") as pmm,
            tc.tile_pool(name="pout", bufs=2, space="PSUM") as pout,
        ):
            # resident weights
            w1r = wp.tile([32 + D, H], bf16)
            nc.sync.dma_start(w1r[:], w1r_d)
            b1p = wp.tile([H, 1], f32)
            nc.sync.dma_start(b1p[:], b1p_d)
            b2 = wp.tile([H, 1], f32)
            nc.sync.dma_start(b2[:], b2_d)
            b3p = wp.tile([H, 1], f32)
            nc.sync.dma_start(b3p[:], b3p_d)
            c0 = wp.tile([H, 1], f32)
            nc.sync.dma_start(c0[:], c0_d)
            w2 = wp.tile([H, H], bf16)
            nc.sync.dma_start(w2[:], w2_d)
            cn = wp.tile([H, H], bf16)
            nc.sync.dma_start(cn[:], cn_d)
            w3 = wp.tile([H, D], bf16)
            nc.sync.dma_start(w3[:], w3_d)
            ones = wp.tile([H, 1], bf16)
            nc.sync.dma_start(ones[:], ones_d)

            for tg in range(NTILES // GROUP):
                t0 = tg * GROUP
                colset = [bass.ts(t0 + j, FD) for j in range(GROUP)]

                # paired z load: tile t0 at partitions 0-7, t0+1 at 32-39
                ztp = iop.tile([32 + D, FD], bf16, tag="zt")
                for j in range(GROUP):
                    nc.sync.dma_start(ztp[32 * j:32 * j + D, :],
                                      zt_d[:, colset[j]])

                # layer-1 matmuls of both tiles, packed in row groups 0/1
                a1s = []
                for j in range(GROUP):
                    a1 = pmm.tile([H, FD], f32, tag="mm", name=f"a1_{tg}_{j}")
                    nc.tensor.matmul(a1[:], w1r[32 * j:32 * j + D, :],
                                     ztp[32 * j:32 * j + D, :],
                                     tile_position=(32 * j, 0))
                    a1s.append(a1)

                h1s, h1sqs = [], []
                for j in range(GROUP):
                    h1 = ap_.tile([H, FD], bf16, tag="h1", name=f"h1_{tg}_{j}")
                    nc.scalar.activation(h1[:], a1s[j][:], Tanh, bias=b1p[:])
                    h1s.append(h1)
                for j in range(GROUP):
                    h1sq = ap_.tile([H, FD], bf16, tag="h1sq",
                                    name=f"h1sq_{tg}_{j}")
                    nc.vector.tensor_tensor(h1sq[:], h1s[j][:], h1s[j][:],
                                            mult)
                    h1sqs.append(h1sq)

                a2s, ss = [], []
                for j in range(GROUP):
                    a2 = pmm.tile([H, FD], f32, tag="mm", name=f"a2_{tg}_{j}")
                    nc.tensor.matmul(a2[:], w2[:], h1s[j][:])
                    a2s.append(a2)
                    s = pmm.tile([H, FD], f32, tag="mm", name=f"s_{tg}_{j}")
                    nc.tensor.matmul(s[:], cn[:], h1sqs[j][:])
                    ss.append(s)

                h2s = []
                for j in range(GROUP):
                    h2 = ap_.tile([H, FD], bf16, tag="h2", name=f"h2_{tg}_{j}")
                    nc.scalar.activation(h2[:], a2s[j][:], Tanh, bias=b2[:])
                    h2s.append(h2)

                ps = []
                for j in range(GROUP):
                    h2sq = ap_.tile([H, FD], bf16, tag="h2sq",
                                    name=f"h2sq_{tg}_{j}")
                    nc.scalar.activation(h2sq[:], h2s[j][:], Square)
                    v = ap_.tile([H, FD], bf16, tag="v", name=f"v_{tg}_{j}")
                    nc.vector.tensor_scalar(v[:], h2sq[:], -1.0, 1.0, mult,
                                            add)
                    p = ap_.tile([H, FD], bf16, tag="p", name=f"p_{tg}_{j}")
                    nc.vector.scalar_tensor_tensor(p[:], ss[j][:], c0[:],
                                                   v[:], add, mult)
                    ps.append(p)

                out_bank = pout.tile([H, FD], f32, tag="ob")
                for j in range(GROUP):
                    nc.tensor.matmul(out_bank[32 * j:32 * j + D, :], w3[:],
                                     h2s[j][:], tile_position=(0, 32 * j))
                    nc.tensor.matmul(out_bank[64 + 32 * j:64 + 32 * j + 1, :],
                                     ones[:], ps[j][:],
                                     tile_position=(0, 64 + 32 * j))

                osb = iop.tile([H, FD], f32, tag="osb")
                if with_b3:
                    nc.vector.tensor_scalar(osb[:], out_bank[:], b3p[:], None,
                                            add)
                else:
                    nc.vector.tensor_copy(osb[:], out_bank[:])
                for j in range(GROUP):
                    nc.sync.dma_start(dzt_d[:, colset[j]],
                                      osb[32 * j:32 * j + D, :])
                    nc.sync.dma_start(dlp_d[:, colset[j]],
                                      osb[64 + 32 * j:64 + 32 * j + 1, :])
    nc.compile()
    return nc


_NC_CACHE = {}


def kernel(z, logp_z, t, W1, b1, W2, b2, W3, b3):
    global LAST_RESULTS, _NC_CACHE

    z = np.ascontiguousarray(np.asarray(z, dtype=np.float32))
    t_s = float(np.asarray(t, dtype=np.float32).reshape(-1)[0])
    W1 = np.asarray(W1, dtype=np.float32)
    b1 = np.asarray(b1, dtype=np.float32)
    W2 = np.asarray(W2, dtype=np.float32)
    b2 = np.asarray(b2, dtype=np.float32)
    W3 = np.asarray(W3, dtype=np.float32)
    b3 = np.asarray(b3, dtype=np.float32)

    W1z = np.ascontiguousarray(W1[:D, :])              # [D, H]
    b1p = (b1 + t_s * W1[D, :]).reshape(H, 1).astype(np.float32)
    M = W3.astype(np.float64) @ W1z.astype(np.float64)  # [H, H]
    C = W2.astype(np.float64) * M.T                     # [H(j), H(k)]
    c0 = C.sum(axis=0).reshape(H, 1).astype(np.float32)  # C^T @ 1
    w1r = np.zeros((32 + D, H), dtype=BF16)
    for j in range(GROUP):
        w1r[32 * j:32 * j + D, :] = W1z.astype(BF16)
    w2_bf = W2.astype(BF16)
    cn_bf = (-C).astype(np.float32).astype(BF16)
    w3_bf = np.ascontiguousarray(W3.astype(BF16))
    ones_neg = np.full((H, 1), -1.0, dtype=BF16)
    b2v = b2.reshape(H, 1).astype(np.float32)
    b3p = np.zeros((H, 1), dtype=np.float32)
    for jj in range(GROUP):
        b3p[32 * jj:32 * jj + D, 0] = b3
    with_b3 = bool(np.any(b3 != 0))

    if with_b3 not in _NC_CACHE:
        _NC_CACHE[with_b3] = _build_bass(with_b3)
    nc = _NC_CACHE[with_b3]

    in_maps = []
    for c in range(NCORES):
        zt = np.ascontiguousarray(z[c * BC:(c + 1) * BC, :].T.astype(BF16))
        in_maps.append({
            "zt": zt,
            "w1r": w1r,
            "b1p": b1p,
            "b2": b2v,
            "b3p": b3p,
            "c0": c0,
            "w2": w2_bf,
            "cneg": cn_bf,
            "w3": w3_bf,
            "onesneg": ones_neg,
        })

    res = run_bass_kernel_spmd(nc, in_maps, core_ids=list(range(NCORES)))
    LAST_RESULTS = res

    dz = np.empty((B, D), dtype=np.float32)
    dlogp = np.empty((B, 1), dtype=np.float32)
    for c in range(NCORES):
        dz[c * BC:(c + 1) * BC, :] = res.results[c]["dzt"].T
        dlogp[c * BC:(c + 1) * BC, 0] = res.results[c]["dlp"].reshape(-1)
    return (dz, dlogp)


# revision 6
# speedup vs baseline: 1.6000x; 1.0707x over previous
"""Trainium2 Bass kernel for a CNF (FFJORD-style) dynamics step.

Computes, for each sample z_i of a batch B=131072 (D=8, H=128):
    x  = concat([z_i, t])
    h1 = tanh(x @ W1 + b1)
    h2 = tanh(h1 @ W2 + b2)
    dz_dt   = h2 @ W3 + b3
    dlogp   = -trace(d dz_dt / d z_i)

The Jacobian trace has the closed form (u = 1-h1^2, v = 1-h2^2):
    trace = v . (u @ C)   with C[j,k] = W2[j,k] * (W3 @ W1[:D])[k,j]
so a single extra HxH matmul per sample replaces the full Jacobian.
On device the "1 -" is folded into the matmul: s'' = (-C)^T h1sq and
p = (s'' + c0) * v with c0 = C^T 1 applied as a fused per-partition
scalar in one DVE scalar_tensor_tensor op.

Sharding: pure data parallel over 8 NeuronCores (batch split).
Layout on device is feature-major ([feature, batch] in SBUF partitions);
the host transposes z per shard and transposes dz_dt back.

Tiles are processed in pairs: layer-1 matmuls of the two tiles run
concurrently in different 32-row groups of the PE array, activations and
elementwise ops run once per pair at free-dim 1024 to amortize per-op
constants, and the dz (M=8) / dlogp (M=1) matmuls of both tiles pack
into the four 32-col groups of one shared PSUM output bank.
"""

import numpy as np
import ml_dtypes

import concourse.bass as bass
import concourse.tile as tile
from concourse import bacc, mybir
from concourse.bass_utils import run_bass_kernel_spmd

BF16 = ml_dtypes.bfloat16

B = 131072
D = 8
H = 128
NCORES = 8
BC = B // NCORES          # samples per core
FD = 512                  # tile free-dim (samples per tile)
NTILES = BC // FD         # 32
GROUP = 2                 # tiles per pair-group
NG = NTILES // GROUP

# bf16 weights packed as one [128, WCOLS] image: w2 | cneg | w3 | ones | w1r
WC_W2 = 0
WC_CN = H
WC_W3 = 2 * H
WC_ON = 2 * H + D
WC_W1 = 2 * H + D + 1
WCOLS = WC_W1 + H

# test.py can read profiling info from here after calling kernel()
LAST_RESULTS = None


def _build_bass(with_b3):
    nc = bacc.Bacc("TRN2", target_bir_lowering=False, debug=False,
                   num_devices=NCORES)
    f32 = mybir.dt.float32
    bf16 = mybir.dt.bfloat16
    FD2 = FD * GROUP

    zt_d = nc.dram_tensor("zt", [D, BC], bf16, kind="ExternalInput").ap()
    wb_d = nc.dram_tensor("wbig", [H, WCOLS], bf16, kind="ExternalInput").ap()
    bias_d = nc.dram_tensor("biases", [H, 4], f32, kind="ExternalInput").ap()

    dzt_d = nc.dram_tensor("dzt", [D, BC], f32, kind="ExternalOutput").ap()
    dlp_d = nc.dram_tensor("dlp", [1, BC], f32, kind="ExternalOutput").ap()

    mult = mybir.AluOpType.mult
    add = mybir.AluOpType.add
    Tanh = mybir.ActivationFunctionType.Tanh
    Square = mybir.ActivationFunctionType.Square

    with tile.TileContext(nc) as tc:
        with (
            tc.tile_pool(name="wts", bufs=1) as wp,
            tc.tile_pool(name="io", bufs=6) as iop,
            tc.tile_pool(name="act", bufs=4) as ap_,
            tc.tile_pool(name="pa1", bufs=1, space="PSUM") as pa1,
            tc.tile_pool(name="pa2", bufs=1, space="PSUM") as pa2,
            tc.tile_pool(name="psm", bufs=2, space="PSUM") as psm,
            tc.tile_pool(name="pout", bufs=2, space="PSUM") as pout,
        ):
            wb = wp.tile([H, WCOLS], bf16)
            nc.sync.dma_start(wb[:], wb_d)
            biases = wp.tile([H, 4], f32)
            nc.sync.dma_start(biases[:], bias_d)
            w2 = wb[:, WC_W2:WC_W2 + H]
            cn = wb[:, WC_CN:WC_CN + H]
            w3 = wb[:, WC_W3:WC_W3 + D]
            ones = wb[:, WC_ON:WC_ON + 1]
            w1r = wb[:, WC_W1:WC_W1 + H]   # rows 0-7 / 32-39 hold W1z
            b1p = biases[:, 0:1]
            b2 = biases[:, 1:2]
            b3p = biases[:, 2:3]
            c0 = biases[:, 3:4]

            for tg in range(NG):
                t0 = tg * GROUP
                colset = [bass.ts(t0 + j, FD) for j in range(GROUP)]
                pcols = bass.ts(tg, FD2)

                # paired z load: tile t0 at partitions 0-7, t0+1 at 32-39
                ztp = iop.tile([32 + D, FD], bf16, tag="zt")
                for j in range(GROUP):
                    nc.sync.dma_start(ztp[32 * j:32 * j + D, :],
                                      zt_d[:, colset[j]])

                # layer-1 matmuls packed in row groups 0/1, one 2-bank out
                a1p = pa1.tile([H, FD2], f32, tag="a1")
                for j in range(GROUP):
                    nc.tensor.matmul(a1p[:, bass.ts(j, FD)],
                                     w1r[32 * j:32 * j + D, :],
                                     ztp[32 * j:32 * j + D, :],
                                     tile_position=(32 * j, 0))

                h1p = ap_.tile([H, FD2], bf16, tag="h1")
                nc.scalar.activation(h1p[:], a1p[:], Tanh, bias=b1p)
                h1sqp = ap_.tile([H, FD2], bf16, tag="h1sq")
                nc.vector.tensor_tensor(h1sqp[:], h1p[:], h1p[:], mult)

                a2p = pa2.tile([H, FD2], f32, tag="a2")
                ss = []
                for j in range(GROUP):
                    nc.tensor.matmul(a2p[:, bass.ts(j, FD)], w2,
                                     h1p[:, bass.ts(j, FD)])
                    s = psm.tile([H, FD], f32, tag="s", name=f"s_{tg}_{j}")
                    nc.tensor.matmul(s[:], cn, h1sqp[:, bass.ts(j, FD)])
                    ss.append(s)

                h2p = ap_.tile([H, FD2], bf16, tag="h2")
                nc.scalar.activation(h2p[:], a2p[:], Tanh, bias=b2)
                h2sqp = ap_.tile([H, FD2], bf16, tag="h2sq")
                nc.scalar.activation(h2sqp[:], h2p[:], Square)
                vp = ap_.tile([H, FD2], bf16, tag="v")
                nc.vector.tensor_scalar(vp[:], h2sqp[:], -1.0, 1.0, mult, add)

                ps = []
                for j in range(GROUP):
                    p = ap_.tile([H, FD], bf16, tag="p", name=f"p_{tg}_{j}")
                    nc.vector.scalar_tensor_tensor(p[:], ss[j][:], c0,
                                                   vp[:, bass.ts(j, FD)],
                                                   add, mult)
                    ps.append(p)

                out_bank = pout.tile([H, FD], f32, tag="ob")
                for j in range(GROUP):
                    nc.tensor.matmul(out_bank[32 * j:32 * j + D, :], w3,
                                     h2p[:, bass.ts(j, FD)],
                                     tile_position=(0, 32 * j))
                    nc.tensor.matmul(out_bank[64 + 32 * j:64 + 32 * j + 1, :],
                                     ones, ps[j][:],
                                     tile_position=(0, 64 + 32 * j))

                osb = iop.tile([H, FD], f32, tag="osb")
                if with_b3:
                    nc.vector.tensor_scalar(osb[:], out_bank[:], b3p, None,
                                            add)
                else:
                    nc.vector.tensor_copy(osb[:], out_bank[:])
                for j in range(GROUP):
                    nc.sync.dma_start(dzt_d[:, colset[j]],
                                      osb[32 * j:32 * j + D, :])
                    nc.sync.dma_start(dlp_d[:, colset[j]],
                                      osb[64 + 32 * j:64 + 32 * j + 1, :])
    nc.compile()
    return nc


_NC_CACHE = {}


def kernel(z, logp_z, t, W1, b1, W2, b2, W3, b3):
    global LAST_RESULTS, _NC_CACHE

    z = np.ascontiguousarray(np.asarray(z, dtype=np.float32))
    t_s = float(np.asarray(t, dtype=np.float32).reshape(-1)[0])
    W1 = np.asarray(W1, dtype=np.float32)
    b1 = np.asarray(b1, dtype=np.float32)
    W2 = np.asarray(W2, dtype=np.float32)
    b2 = np.asarray(b2, dtype=np.float32)
    W3 = np.asarray(W3, dtype=np.float32)
    b3 = np.asarray(b3, dtype=np.float32)

    W1z = np.ascontiguousarray(W1[:D, :])              # [D, H]
    b1p = (b1 + t_s * W1[D, :]).astype(np.float32)
    M = W3.astype(np.float64) @ W1z.astype(np.float64)  # [H, H]
    C = W2.astype(np.float64) * M.T                     # [H(j), H(k)]
    c0 = C.sum(axis=0).astype(np.float32)               # C^T @ 1

    wbig = np.zeros((H, WCOLS), dtype=BF16)
    wbig[:, WC_W2:WC_W2 + H] = W2.astype(BF16)
    wbig[:, WC_CN:WC_CN + H] = (-C).astype(np.float32).astype(BF16)
    wbig[:, WC_W3:WC_W3 + D] = W3.astype(BF16)
    wbig[:, WC_ON] = -1.0
    for j in range(GROUP):
        wbig[32 * j:32 * j + D, WC_W1:WC_W1 + H] = W1z.astype(BF16)

    biases = np.zeros((H, 4), dtype=np.float32)
    biases[:, 0] = b1p
    biases[:, 1] = b2
    for jj in range(GROUP):
        biases[32 * jj:32 * jj + D, 2] = b3
    biases[:, 3] = c0
    with_b3 = bool(np.any(b3 != 0))

    if with_b3 not in _NC_CACHE:
        _NC_CACHE[with_b3] = _build_bass(with_b3)
    nc = _NC_CACHE[with_b3]

    in_maps = []
    for c in range(NCORES):
        zt = np.ascontiguousarray(z[c * BC:(c + 1) * BC, :].T.astype(BF16))
        in_maps.append({"zt": zt, "wbig": wbig, "biases": biases})

    res = run_bass_kernel_spmd(nc, in_maps, core_ids=list(range(NCORES)))
    LAST_RESULTS = res

    dz = np.empty((B, D), dtype=np.float32)
    dlogp = np.empty((B, 1), dtype=np.float32)
    for c in range(NCORES):
        dz[c * BC:(c + 1) * BC, :] = res.results[c]["dzt"].T
        dlogp[c * BC:(c + 1) * BC, 0] = res.results[c]["dlp"].reshape(-1)
    return (dz, dlogp)
